# revision 33
# baseline (speedup 1.0000x reference)
"""AttentionOnAttention Trainium2 kernel (8 NeuronCores, SPMD), v3.

Sharding: core c handles batch b = c//4 and heads [4*(c%4), 4*(c%4)+4);
each core computes the disjoint output slice out[b, :, 256*(c%4):...] so no
collectives are needed.

Measured 212.4us (vs 213.3us baseline reproduction; rel err 4.1e-3).
Structure: ~11us DMA+mini-prefix (q/k chunk0 + v tiles 0-3), then ONE
128-step exp stream at ~1013ns/step (ACT-paced; HAM stays K=8/8 warm the
whole run), then a ~15us tail. All projections ride the stream as
deadline-ordered PE fillers; per-chunk normalize rides chunk n+1 on even
steps and finals (AoA+gate+output DMA) ride chunk n+2 on odd steps.

Key mechanisms (measured, do not regress):
  - augmented full-array PV: per pair two stationary tiles
    augA=[v_A|1|z63], augB=[z32|1|z31|v_B] (128 weight cols -> FWL, ~2
    cols/cycle; full-array K=128 bf16 matmuls stream 2x vs row/col-tiled).
    Bank A rows 0:64=ao_A row 64=L_A; bank B row 32=L_B rows 64:128=ao_B.
    L rows must land on 32-aligned partitions (PSUM read rule). Two banks
    because start=True clears has_written for a whole bank (one
    accumulation group per bank).
  - tanh-form gate: sigmoid(g)=0.5(1+tanh(g/2)) exactly, with I-columns of
    wcq/wca and both biases pre-halved host-side -> two STTs on DVE, and
    NO ACT table switch anywhere (exp+tanh share one set).
  - two-deep interleaved carry: every boundary item enters its engine
    queue >=2 steps after its producers; a not-ready tanh in the in-order
    ACT queue stalls ALL later exps (was +130us when finals rode n+1).
  - emission order IS dependency order for the tile framework: ensure()
    force-drains filler units before their consumers are emitted.
  - DMA priority order with tiny bias/wcqa dispatches first (early ACT
    table load); first exp at ~11us.

Dead ends, measured (do not resurrect):
  - DVE fp16 acc for softmax denominators: +83us DVE, both DVE and PE
    saturate -> 227us. The in-matmul ones-column L is free.
  - acc split DVE/gpsimd: gpsimd tensor_add [128,384] is 0.9-1.7us ->
    paces the stream -> 243us. gpsimd is only good for partition_broadcast
    at boundaries.
  - finals A/B interleaved IN-stream: ig PSUM tiles then live ~8 steps in
    the 3-slot spair ring -> S matmuls stall -> 230us.
  - popping fillers before the step's S pair: +1us (delays the ACT pacer).
  - fp8 DoubleRow (prev session): HAM activity-class 1 (50% cap) -> 341us.
  - tile_position col-packed PV [64,512]x2 at (0,0)/(0,64): works (pair
    ~405ns) but 1 col/cycle; the full-array augmented form is faster AND
    gives L for free.
  - exp without max-subtraction is safe (|S*scale| < ~6).
  - reciprocal_approx_fast misbehaves at partition offsets != 0; L is
    copied to partition 0 first (DVE copies may shift partitions).
"""

import numpy as np
from collections import deque
from contextlib import ExitStack

import concourse.bass as bass
import concourse.bacc as bacc
import concourse.tile as tile
from concourse import mybir

B, N, DIM, H, DH = 2, 2048, 1024, 16, 64
HPC = H // 4          # 4 heads per core
INC = HPC * DH        # 256 per-core inner width
KT = DIM // 128       # 8 contraction tiles
NCH = N // 512        # 4 free-dim chunks of 512
JT = N // 128         # 16 j tiles
SCALE = float(DH) ** -0.5
F32 = mybir.dt.float32
F16 = mybir.dt.float16
BF16 = mybir.dt.bfloat16
AF = mybir.ActivationFunctionType
ALU = mybir.AluOpType

# filler pacing: ns of estimated PE filler cost admitted per stream step.
FILL_EARLY = 1000     # during chunk (p0,c0): ACT is still ramping anyway
FILL = 350


def build_nc():
    nc = bacc.Bacc(
        "TRN2",
        target_bir_lowering=False,
        debug=False,
        enable_asserts=False,
        num_devices=8,
    )
    xT_d = nc.dram_tensor("xT", (KT, 128, N), BF16, kind="ExternalInput").ap()
    wqkv_d = nc.dram_tensor("wqkv", (DIM, 3 * INC), BF16, kind="ExternalInput").ap()
    wcq_d = nc.dram_tensor("wcq", (DH, 2 * DH), BF16, kind="ExternalInput").ap()
    wca_d = nc.dram_tensor("wca", (DH, 2 * DH), BF16, kind="ExternalInput").ap()
    bias_d = nc.dram_tensor("biases", (2 * DH, 1), F32, kind="ExternalInput").ap()
    outT_d = nc.dram_tensor("outT", (INC, N), BF16, kind="ExternalOutput").ap()

    with tile.TileContext(nc) as tc, ExitStack() as ctx:
        consts = ctx.enter_context(tc.tile_pool(name="consts", bufs=1))
        psum = ctx.enter_context(tc.tile_pool(name="psum", bufs=3, space="PSUM"))

        # persistent per-chunk tiles (precise dependency granularity)
        qTc = [[consts.tile([128, 512], BF16, name=f"qT{p}_{c}") for c in range(NCH)]
               for p in range(2)]
        kTc = [[consts.tile([128, 512], BF16, name=f"kT{p}_{c}") for c in range(NCH)]
               for p in range(2)]
        # varena[it]: per pair two augmented stationary tiles with 128 weight
        # cols each (FWL-eligible full-array matmuls stream ~2 cols/cycle):
        #   aug0 = [v_{2p} | 1 | zeros_63]  -> PV rows 0:64 = ao_A, row 64 = L_A
        #   aug1 = [z32 | 1 | z31 | v_{2p+1}] -> PV row 32 = L_B, rows 64:128 = ao_B
        varena = [consts.tile([128, 2, 2, 128], BF16, name=f"v{it}")
                  for it in range(JT)]
        for it in range(JT):
            for p2 in range(2):
                nc.vector.memset(varena[it][:, p2, 0, DH:128], 0.0)
                nc.vector.memset(varena[it][:, p2, 0, DH : DH + 1], 1.0)
                nc.vector.memset(varena[it][:, p2, 1, 0:DH], 0.0)
                nc.vector.memset(varena[it][:, p2, 1, 32:33], 1.0)
        # qac[h][c]: rows 0:64 = q_h^T, rows 64:128 = normalized ao_h^T
        qac = [[consts.tile([128, 512], BF16, name=f"qa{h}_{c}") for c in range(NCH)]
               for h in range(HPC)]

        esp = ctx.enter_context(tc.tile_pool(name="es_p", bufs=8))
        aop = ctx.enter_context(tc.tile_pool(name="ao_p", bufs=2))
        nrm = ctx.enter_context(tc.tile_pool(name="norm_p", bufs=4))
        fin = ctx.enter_context(tc.tile_pool(name="fin_p", bufs=6))

        xw = ctx.enter_context(tc.tile_pool(name="xw", bufs=1))
        wqkv_sb = xw.tile([128, KT, 3 * INC], BF16, name="wqkv_sb")
        wq_sb = wqkv_sb[:, :, 0:INC]
        wk_sb = wqkv_sb[:, :, INC : 2 * INC]
        wv_sb = wqkv_sb[:, :, 2 * INC : 3 * INC]
        xt_sb = xw.tile([128, KT, N], BF16, name="xt_sb")

        # ---- DMA dispatch order = criticality order ----
        wqkv_r = wqkv_d.rearrange("(kt p) c -> p kt c", p=128)
        xT_r = xT_d.rearrange("k p n -> p k n")
        # tiny first: biases + AoA weights (lets ACT warm its table early)
        bias_sb = consts.tile([2 * DH, 1], F32, name="bias_sb")
        nc.sync.dma_start(out=bias_sb, in_=bias_d)
        wcqa_sb = consts.tile([128, 2 * DH], BF16, name="wcqa_sb")
        nc.sync.dma_start(out=wcqa_sb[0:DH, :], in_=wcq_d)
        nc.sync.dma_start(out=wcqa_sb[DH:128, :], in_=wca_d)
        # q/k weights pair 0, x chunk 0 -> the critical prefix inputs
        nc.sync.dma_start(out=wqkv_sb[:, :, 0:128], in_=wqkv_r[:, :, 0:128])
        nc.sync.dma_start(out=wqkv_sb[:, :, INC : INC + 128],
                          in_=wqkv_r[:, :, INC : INC + 128])
        nc.sync.dma_start(out=xt_sb[:, :, 0:512], in_=xT_r[:, :, 0:512])
        nc.sync.dma_start(out=wqkv_sb[:, :, 2 * INC : 3 * INC],
                          in_=wqkv_r[:, :, 2 * INC : 3 * INC])
        nc.sync.dma_start(out=xt_sb[:, :, 512:1024], in_=xT_r[:, :, 512:1024])
        nc.sync.dma_start(out=xt_sb[:, :, 1024:1536], in_=xT_r[:, :, 1024:1536])
        nc.sync.dma_start(out=xt_sb[:, :, 1536:2048], in_=xT_r[:, :, 1536:2048])
        nc.sync.dma_start(out=wqkv_sb[:, :, 128:INC], in_=wqkv_r[:, :, 128:INC])
        nc.sync.dma_start(out=wqkv_sb[:, :, INC + 128 : 2 * INC],
                          in_=wqkv_r[:, :, INC + 128 : 2 * INC])

        # ACT table warmup: exp+tanh share one set; nothing else is used.
        warm_sb = consts.tile([128, 1], F32, name="warm_sb")
        nc.scalar.activation(out=warm_sb, in_=bias_sb, func=AF.Exp)
        nc.scalar.activation(out=warm_sb, in_=warm_sb, func=AF.Tanh)

        # ---------------- projection units (as filler micros) ----------------
        def qk_micros(wsb, p, c, is_q):
            """8 matmuls + evacuation, one closure per instruction group."""
            state = {}

            def mk_mm(k):
                def _mm():
                    if "ps" not in state:
                        state["ps"] = psum.tile(
                            [128, 512], F32, name="ps_qk", tag="spair", bufs=3
                        )
                    nc.tensor.matmul(
                        state["ps"],
                        lhsT=wsb[:, k, p * 128 : (p + 1) * 128],
                        rhs=xt_sb[:, k, c * 512 : (c + 1) * 512],
                        start=(k == 0),
                        stop=(k == KT - 1),
                    )
                return (260, _mm)

            def _evac_pair():
                dst = qTc[p][c] if is_q else kTc[p][c]
                nc.vector.tensor_copy(out=dst, in_=state["ps"])

            if not is_q:
                return [mk_mm(k) for k in range(KT)] + [(60, _evac_pair)]

            def _evac_qa():
                ps = state["ps"]
                nc.vector.tensor_copy(out=qac[2 * p][c][0:DH, :], in_=ps[0:DH, :])
                nc.vector.tensor_copy(out=qac[2 * p + 1][c][0:DH, :],
                                      in_=ps[DH:128, :])

            return [mk_mm(k) for k in range(KT)] + [(60, _evac_pair),
                                                    (60, _evac_qa)]

        def v_micros(it):
            state = {}

            def mk_mm(k):
                def _mm():
                    if "ps" not in state:
                        state["ps"] = psum.tile(
                            [128, 512], F32, name="ps_v", tag="spair", bufs=3
                        )
                    nc.tensor.matmul(
                        state["ps"][:, 0:INC],
                        lhsT=xt_sb[:, k, it * 128 : (it + 1) * 128],
                        rhs=wv_sb[:, k, :],
                        start=(k == 0),
                        stop=(k == KT - 1),
                    )
                return (150, _mm)

            def _evac0():
                for p2 in range(2):
                    nc.vector.tensor_copy(
                        out=varena[it][:, p2, 0, 0:DH],
                        in_=state["ps"][:, (2 * p2) * DH : (2 * p2 + 1) * DH],
                    )

            def _evac1():
                for p2 in range(2):
                    nc.vector.tensor_copy(
                        out=varena[it][:, p2, 1, DH:128],
                        in_=state["ps"][:, (2 * p2 + 1) * DH : (2 * p2 + 2) * DH],
                    )

            return [mk_mm(k) for k in range(KT)] + [(60, _evac0), (60, _evac1)]

        # ---------------- filler queue (deadline order) ----------------
        # Emission order IS dependency order for the tile framework, so a
        # consumer (S/PV/AoA) must be emitted after its producer unit. The
        # `ensure` calls force-drain fillers up to the labeled unit; the
        # budget metering only paces additional prefetch.
        fillers = deque()
        emitted_units = set()

        def push_unit(ms, label):
            ms = list(ms)
            for cost, fn in ms[:-1]:
                fillers.append((cost, fn, None))
            cost, fn = ms[-1]
            fillers.append((cost, fn, label))

        def pop_filler():
            cost, fn, label = fillers.popleft()
            fn()
            if label is not None:
                emitted_units.add(label)
            return cost

        def ensure(label):
            while fillers and label not in emitted_units:
                pop_filler()

        # prefix (emitted immediately): q(p0,c0), k(p0,c0), v tiles 0-5
        for cost, fn in qk_micros(wq_sb, 0, 0, True):
            fn()
        for cost, fn in qk_micros(wk_sb, 0, 0, False):
            fn()
        for it in range(4):
            for cost, fn in v_micros(it):
                fn()

        push_unit(v_micros(4), ("v", 4))
        push_unit(v_micros(5), ("v", 5))
        push_unit(qk_micros(wk_sb, 0, 1, False), ("k", 0, 1))
        push_unit(v_micros(6), ("v", 6))
        push_unit(v_micros(7), ("v", 7))
        push_unit(qk_micros(wk_sb, 0, 2, False), ("k", 0, 2))
        push_unit(v_micros(8), ("v", 8))
        push_unit(v_micros(9), ("v", 9))
        push_unit(qk_micros(wk_sb, 0, 3, False), ("k", 0, 3))
        push_unit(qk_micros(wq_sb, 0, 1, True), ("q", 0, 1))
        push_unit(v_micros(10), ("v", 10))
        push_unit(v_micros(11), ("v", 11))
        push_unit(v_micros(12), ("v", 12))
        push_unit(qk_micros(wq_sb, 0, 2, True), ("q", 0, 2))
        push_unit(v_micros(13), ("v", 13))
        push_unit(v_micros(14), ("v", 14))
        push_unit(v_micros(15), ("v", 15))
        push_unit(qk_micros(wq_sb, 0, 3, True), ("q", 0, 3))
        push_unit(qk_micros(wk_sb, 1, 0, False), ("k", 1, 0))
        push_unit(qk_micros(wq_sb, 1, 0, True), ("q", 1, 0))
        push_unit(qk_micros(wk_sb, 1, 1, False), ("k", 1, 1))
        push_unit(qk_micros(wk_sb, 1, 2, False), ("k", 1, 2))
        push_unit(qk_micros(wk_sb, 1, 3, False), ("k", 1, 3))
        push_unit(qk_micros(wq_sb, 1, 1, True), ("q", 1, 1))
        push_unit(qk_micros(wq_sb, 1, 2, True), ("q", 1, 2))
        push_unit(qk_micros(wq_sb, 1, 3, True), ("q", 1, 3))

        # ---------------- per-chunk boundary work ----------------
        def mk_norm(p, c, pv_state, emit_pv):
            """Carry items riding chunk n+1: finish PV, evacuate ao, and
            normalize using the free L rows (pvA row 64 / pvB row 63)."""
            items = []

            def _pvlast():
                ensure(("v", JT - 1))
                emit_pv(JT - 2)
                emit_pv(JT - 1)
            items.append(_pvlast)

            lst = {}

            def mk_evac(hh):
                def _evac():
                    # ao + L row to SBUF at base partition 0 (DVE copies may
                    # shift partitions); frees the pv bank for the next chunk
                    ao = aop.tile([DH, 512], BF16, name=f"ao{hh}",
                                  tag=f"ao{hh}")
                    lst[f"ao{hh}"] = ao
                    l0 = nrm.tile([1, 512], F32, name=f"l0_{hh}",
                                  tag=f"l0{hh}")
                    lst[f"l0{hh}"] = l0
                    if hh == 0:
                        nc.vector.tensor_copy(out=ao,
                                              in_=pv_state["pvA"][0:DH, :])
                        nc.vector.tensor_copy(
                            out=l0, in_=pv_state["pvA"][DH : DH + 1, :])
                    else:
                        nc.vector.tensor_copy(out=ao,
                                              in_=pv_state["pvB"][DH:128, :])
                        nc.vector.tensor_copy(
                            out=l0, in_=pv_state["pvB"][32:33, :])
                return _evac
            items.append(mk_evac(0))
            items.append(mk_evac(1))

            def mk_lrec(hh):
                def _lrec():
                    rl = nrm.tile([1, 512], F32, name="rl", tag=f"rl{hh}")
                    nc.vector.reciprocal_approx_fast(out=rl,
                                                     in_=lst[f"l0{hh}"])
                    rlb = nrm.tile([DH, 512], F32, name="rlb", tag=f"rlb{hh}")
                    lst[f"rlb{hh}"] = rlb
                    nc.gpsimd.partition_broadcast(rlb, rl)
                return _lrec
            items.append(mk_lrec(0))
            items.append(mk_lrec(1))

            def mk_mul(hh):
                def _mul():
                    # normalized ao -> rows 64:128 of qac (DVE)
                    nc.vector.tensor_mul(
                        out=qac[2 * p + hh][c][DH:128, :],
                        in0=lst[f"ao{hh}"],
                        in1=lst[f"rlb{hh}"],
                    )
                return _mul
            items.append(mk_mul(0))
            items.append(mk_mul(1))
            return items

        def mk_finals_items(p, c):
            """Carry items riding chunk n+2: AoA + tanh gate + output."""
            items = []

            def mk_finals(hh):
                h = 2 * p + hh
                cs = slice(c * 512, (c + 1) * 512)
                fst = {}

                def _aoa():
                    fst["ig"] = psum.tile([128, 512], F32, name="igh",
                                          tag="spair", bufs=3)
                    nc.tensor.matmul(fst["ig"], lhsT=wcqa_sb, rhs=qac[h][c],
                                     start=True, stop=True)

                def _tanh():
                    t = fin.tile([DH, 512], BF16, name="tgate", tag="t")
                    fst["t"] = t
                    nc.scalar.activation(
                        out=t, in_=fst["ig"][DH:128, :], func=AF.Tanh,
                        scale=0.5, bias=bias_sb[DH : 2 * DH, :],
                    )

                def _stt1():
                    # P = (0.5*I_raw + 0.5*b_out) * t   (weights/bias pre-halved)
                    pt = fin.tile([DH, 512], BF16, name="pt", tag="pt")
                    fst["pt"] = pt
                    nc.vector.scalar_tensor_tensor(
                        out=pt, in0=fst["ig"][0:DH, :],
                        scalar=bias_sb[0:DH, :], in1=fst["t"],
                        op0=ALU.add, op1=ALU.mult,
                    )

                def _stt2():
                    # out = (0.5*I_raw + 0.5*b_out) + P  = (I+b)*sigmoid(G+bg)
                    ot = fin.tile([DH, 512], BF16, name="ot", tag="ot")
                    nc.vector.scalar_tensor_tensor(
                        out=ot, in0=fst["ig"][0:DH, :],
                        scalar=bias_sb[0:DH, :], in1=fst["pt"],
                        op0=ALU.add, op1=ALU.add,
                    )
                    nc.sync.dma_start(out=outT_d[h * DH : (h + 1) * DH, cs],
                                      in_=ot)

                return [_aoa, _tanh, _stt1, _stt2]

            items.extend(mk_finals(0))
            items.extend(mk_finals(1))
            return items

        # ---------------- the stream ----------------
        carry = []
        pending_finals = []
        chunk_idx = 0
        for p in range(2):
            for c in range(NCH):
                pv_state = {}
                es_tiles = [None] * JT

                def emit_pv(jt, p=p, pv_state=pv_state, es_tiles=es_tiles):
                    # Full-array 128-weight-col matmuls (FWL, ~2 cols/cycle).
                    # Bank A rows 0:64 = ao_A, row 64 = L_A (ones col);
                    # bank B row 63 = L_B, rows 64:128 = ao_B. The zero-
                    # padded weight cols make the other rows harmless.
                    if "pvA" not in pv_state:
                        pv_state["pvA"] = psum.tile([128, 512], F32, name="pvA",
                                                    tag="pv", bufs=2)
                        pv_state["pvB"] = psum.tile([128, 512], F32, name="pvB",
                                                    tag="pv", bufs=2)
                    nc.tensor.matmul(
                        pv_state["pvA"], lhsT=varena[jt][:, p, 0, :],
                        rhs=es_tiles[jt][:, 0:512],
                        start=(jt == 0), stop=(jt == JT - 1),
                    )
                    nc.tensor.matmul(
                        pv_state["pvB"], lhsT=varena[jt][:, p, 1, :],
                        rhs=es_tiles[jt][:, 512:1024],
                        start=(jt == 0), stop=(jt == JT - 1),
                    )

                budget = 0.0
                if (p, c) != (0, 0):
                    ensure(("q", p, c))
                for jt in range(JT):
                    if p > 0 or jt // 4 > 0:
                        ensure(("k", p, jt // 4))
                    jts = slice((jt % 4) * 128, (jt % 4) * 128 + 128)
                    kt_t = kTc[p][jt // 4]
                    s = psum.tile([128, 1024], F32, name="s", tag="spair",
                                  bufs=3)
                    nc.tensor.matmul(
                        s[:, 0:512], lhsT=kt_t[0:DH, jts], rhs=qTc[p][c][0:DH, :],
                        start=True, stop=True, tile_position=(0, 0),
                    )
                    nc.tensor.matmul(
                        s[:, 512:1024], lhsT=kt_t[DH:128, jts],
                        rhs=qTc[p][c][DH:128, :],
                        start=True, stop=True, tile_position=(64, 0),
                    )
                    es = esp.tile([128, 1024], BF16, name="es", tag="es")
                    nc.scalar.activation(out=es, in_=s, func=AF.Exp, scale=SCALE)
                    es_tiles[jt] = es
                    # earlier chunks' boundary work, one item per step:
                    # norm of chunk n-1 (even steps) / finals of chunk n-2
                    # (odd steps) -- interleaved so every chain hop gets ~2
                    # steps of slack and never blocks the in-order PE queue
                    if carry:
                        carry.pop(0)()
                    # PV lags 2 steps so the pv-bank evac of the previous
                    # chunk (carry item at step 2) clears before PV(0) needs
                    # the bank
                    if jt > 1:
                        if jt - 2 > 3:
                            ensure(("v", jt - 2))
                        emit_pv(jt - 2)
                    # projection fillers, cost-metered
                    budget += FILL_EARLY if chunk_idx == 0 else FILL
                    while fillers and fillers[0][0] <= budget:
                        budget -= pop_filler()

                while carry:
                    carry.pop(0)()
                # two-deep carry: norm (pv finish/evac/1/L/qac write) rides
                # the NEXT chunk on even steps; finals (AoA/tanh/output) of
                # the chunk before ride on odd steps. Every item's inputs
                # are >=2 steps old when it enters its engine queue.
                norm_items = mk_norm(p, c, pv_state, emit_pv)
                carry = []
                for i in range(max(len(norm_items), len(pending_finals))):
                    if i < len(norm_items):
                        carry.append(norm_items[i])
                    if i < len(pending_finals):
                        carry.append(pending_finals[i])
                pending_finals = mk_finals_items(p, c)
                chunk_idx += 1

        # ---------------- tail: flush the last chunks ----------------
        while fillers:
            pop_filler()
        while carry:
            carry.pop(0)()
        for it in pending_finals:
            it()
    nc.compile()
    return nc


_NC_CACHE = None


def _get_nc():
    global _NC_CACHE
    if _NC_CACHE is None:
        _NC_CACHE = build_nc()
    return _NC_CACHE


def make_in_maps(x, Wq, Wkv, Wq_out, Wattn_out, out_bias, Wq_gate, Wattn_gate,
                 gate_bias):
    import ml_dtypes

    bf16 = ml_dtypes.bfloat16
    # I-columns (first DH) pre-halved for the tanh-form gate; biases halved.
    wcq = np.concatenate([0.5 * Wq_out.T, Wq_gate.T], axis=1)
    wcq = np.ascontiguousarray(wcq, dtype=bf16)
    wca = np.concatenate([0.5 * Wattn_out.T, Wattn_gate.T], axis=1)
    wca = np.ascontiguousarray(wca, dtype=bf16)
    biases = 0.5 * np.concatenate(
        [out_bias.reshape(-1), gate_bias.reshape(-1)]
    ).astype(np.float32).reshape(2 * DH, 1)
    biases = np.ascontiguousarray(biases)
    Wk = Wkv[:, : H * DH]
    Wv = Wkv[:, H * DH :]
    xT = [
        np.ascontiguousarray(x[b].T.reshape(KT, 128, N)).astype(bf16)
        for b in range(B)
    ]
    in_maps = []
    for c in range(8):
        b, hg = c // 4, c % 4
        cols = slice(hg * INC, (hg + 1) * INC)
        in_maps.append(
            {
                "xT": xT[b],
                "wqkv": np.ascontiguousarray(
                    np.concatenate(
                        [Wq[:, cols], Wk[:, cols], Wv[:, cols]], axis=1
                    )
                ).astype(bf16),
                "wcq": wcq,
                "wca": wca,
                "biases": biases,
            }
        )
    return in_maps


def assemble_output(results):
    out = np.empty((B, N, H * DH), dtype=np.float32)
    for c in range(8):
        b, hg = c // 4, c % 4
        out[b, :, hg * INC : (hg + 1) * INC] = results[c]["outT"].T.astype(np.float32)
    return out


def kernel(**inputs):
    from concourse.bass_utils import run_bass_kernel_spmd

    inputs = {k: np.asarray(v, dtype=np.float32) for k, v in inputs.items()}
    nc = _get_nc()
    in_maps = make_in_maps(**inputs)
    res = run_bass_kernel_spmd(nc, in_maps, core_ids=list(range(8)))
    return assemble_output(res.results)


# revision 35
# speedup vs baseline: 1.0200x; 1.0200x over previous
"""AttentionOnAttention Trainium2 kernel (8 NeuronCores, SPMD), v3.

Sharding: core c handles batch b = c//4 and heads [4*(c%4), 4*(c%4)+4);
each core computes the disjoint output slice out[b, :, 256*(c%4):...] so no
collectives are needed.

Measured 212.4us (vs 213.3us baseline reproduction; rel err 4.1e-3).
Structure: ~11us DMA+mini-prefix (q/k chunk0 + v tiles 0-3), then ONE
128-step exp stream at ~1013ns/step (ACT-paced; HAM stays K=8/8 warm the
whole run), then a ~15us tail. All projections ride the stream as
deadline-ordered PE fillers; per-chunk normalize rides chunk n+1 on even
steps and finals (AoA+gate+output DMA) ride chunk n+2 on odd steps.

Key mechanisms (measured, do not regress):
  - augmented full-array PV: per pair two stationary tiles
    augA=[v_A|1|z63], augB=[z32|1|z31|v_B] (128 weight cols -> FWL, ~2
    cols/cycle; full-array K=128 bf16 matmuls stream 2x vs row/col-tiled).
    Bank A rows 0:64=ao_A row 64=L_A; bank B row 32=L_B rows 64:128=ao_B.
    L rows must land on 32-aligned partitions (PSUM read rule). Two banks
    because start=True clears has_written for a whole bank (one
    accumulation group per bank).
  - tanh-form gate: sigmoid(g)=0.5(1+tanh(g/2)) exactly, with I-columns of
    wcq/wca and both biases pre-halved host-side -> two STTs on DVE, and
    NO ACT table switch anywhere (exp+tanh share one set).
  - two-deep interleaved carry: every boundary item enters its engine
    queue >=2 steps after its producers; a not-ready tanh in the in-order
    ACT queue stalls ALL later exps (was +130us when finals rode n+1).
  - emission order IS dependency order for the tile framework: ensure()
    force-drains filler units before their consumers are emitted.
  - DMA priority order with tiny bias/wcqa dispatches first (early ACT
    table load); first exp at ~11us.

Dead ends, measured (do not resurrect):
  - DVE fp16 acc for softmax denominators: +83us DVE, both DVE and PE
    saturate -> 227us. The in-matmul ones-column L is free.
  - acc split DVE/gpsimd: gpsimd tensor_add [128,384] is 0.9-1.7us ->
    paces the stream -> 243us. gpsimd is only good for partition_broadcast
    at boundaries.
  - finals A/B interleaved IN-stream: ig PSUM tiles then live ~8 steps in
    the 3-slot spair ring -> S matmuls stall -> 230us.
  - popping fillers before the step's S pair: +1us (delays the ACT pacer);
    also true when restricted to the first-2-chunk production wall (+1.4us)
    - the wall is gated by DMA arrival + k-evac chains, not queue order.
  - pipelined (A/B-transposed) tail flush of the last finals: no gain
    within noise. Run-to-run variance of this exact kernel: 212.4-214.1us
    (HAM phase jitter); quote the best-of-3 when comparing changes.
  - fp8 DoubleRow (prev session): HAM activity-class 1 (50% cap) -> 341us.
  - tile_position col-packed PV [64,512]x2 at (0,0)/(0,64): works (pair
    ~405ns) but 1 col/cycle; the full-array augmented form is faster AND
    gives L for free.
  - exp without max-subtraction is safe (|S*scale| < ~6).
  - reciprocal_approx_fast misbehaves at partition offsets != 0; L is
    copied to partition 0 first (DVE copies may shift partitions).
"""

import numpy as np
from collections import deque
from contextlib import ExitStack

import concourse.bass as bass
import concourse.bacc as bacc
import concourse.tile as tile
from concourse import mybir

B, N, DIM, H, DH = 2, 2048, 1024, 16, 64
HPC = H // 4          # 4 heads per core
INC = HPC * DH        # 256 per-core inner width
KT = DIM // 128       # 8 contraction tiles
NCH = N // 512        # 4 free-dim chunks of 512
JT = N // 128         # 16 j tiles
SCALE = float(DH) ** -0.5
F32 = mybir.dt.float32
F16 = mybir.dt.float16
BF16 = mybir.dt.bfloat16
AF = mybir.ActivationFunctionType
ALU = mybir.AluOpType

# filler pacing: ns of estimated PE filler cost admitted per stream step.
FILL_EARLY = 1000     # during chunk (p0,c0): ACT is still ramping anyway
FILL = 350


def build_nc():
    nc = bacc.Bacc(
        "TRN2",
        target_bir_lowering=False,
        debug=False,
        enable_asserts=False,
        num_devices=8,
    )
    xT_d = nc.dram_tensor("xT", (KT, 128, N), BF16, kind="ExternalInput").ap()
    wqkv_d = nc.dram_tensor("wqkv", (DIM, 3 * INC), BF16, kind="ExternalInput").ap()
    wcq_d = nc.dram_tensor("wcq", (DH, 2 * DH), BF16, kind="ExternalInput").ap()
    wca_d = nc.dram_tensor("wca", (DH, 2 * DH), BF16, kind="ExternalInput").ap()
    bias_d = nc.dram_tensor("biases", (2 * DH, 1), F32, kind="ExternalInput").ap()
    outT_d = nc.dram_tensor("outT", (INC, N), BF16, kind="ExternalOutput").ap()

    with tile.TileContext(nc) as tc, ExitStack() as ctx:
        consts = ctx.enter_context(tc.tile_pool(name="consts", bufs=1))
        psum = ctx.enter_context(tc.tile_pool(name="psum", bufs=3, space="PSUM"))

        # persistent per-chunk tiles (precise dependency granularity)
        qTc = [[consts.tile([128, 512], BF16, name=f"qT{p}_{c}") for c in range(NCH)]
               for p in range(2)]
        kTc = [[consts.tile([128, 512], BF16, name=f"kT{p}_{c}") for c in range(NCH)]
               for p in range(2)]
        # varena[it]: per pair two augmented stationary tiles with 128 weight
        # cols each (FWL-eligible full-array matmuls stream ~2 cols/cycle):
        #   aug0 = [v_{2p} | 1 | zeros_63]  -> PV rows 0:64 = ao_A, row 64 = L_A
        #   aug1 = [z32 | 1 | z31 | v_{2p+1}] -> PV row 32 = L_B, rows 64:128 = ao_B
        varena = [consts.tile([128, 2, 2, 128], BF16, name=f"v{it}")
                  for it in range(JT)]
        for it in range(JT):
            for p2 in range(2):
                nc.vector.memset(varena[it][:, p2, 0, DH:128], 0.0)
                nc.vector.memset(varena[it][:, p2, 0, DH : DH + 1], 1.0)
                nc.vector.memset(varena[it][:, p2, 1, 0:DH], 0.0)
                nc.vector.memset(varena[it][:, p2, 1, 32:33], 1.0)
        # qac[h][c]: rows 0:64 = q_h^T, rows 64:128 = normalized ao_h^T
        qac = [[consts.tile([128, 512], BF16, name=f"qa{h}_{c}") for c in range(NCH)]
               for h in range(HPC)]

        esp = ctx.enter_context(tc.tile_pool(name="es_p", bufs=8))
        aop = ctx.enter_context(tc.tile_pool(name="ao_p", bufs=2))
        nrm = ctx.enter_context(tc.tile_pool(name="norm_p", bufs=4))
        fin = ctx.enter_context(tc.tile_pool(name="fin_p", bufs=6))

        xw = ctx.enter_context(tc.tile_pool(name="xw", bufs=1))
        wqkv_sb = xw.tile([128, KT, 3 * INC], BF16, name="wqkv_sb")
        wq_sb = wqkv_sb[:, :, 0:INC]
        wk_sb = wqkv_sb[:, :, INC : 2 * INC]
        wv_sb = wqkv_sb[:, :, 2 * INC : 3 * INC]
        xt_sb = xw.tile([128, KT, N], BF16, name="xt_sb")

        # ---- DMA dispatch order = criticality order ----
        wqkv_r = wqkv_d.rearrange("(kt p) c -> p kt c", p=128)
        xT_r = xT_d.rearrange("k p n -> p k n")
        # bias first (the ACT warmup + table load gate the first exp);
        # then the critical prefix inputs. wcq/wca demoted to the end:
        # finals need them only ~20 steps into the stream.
        bias_sb = consts.tile([2 * DH, 1], F32, name="bias_sb")
        nc.sync.dma_start(out=bias_sb, in_=bias_d)
        # q/k weights pair 0, x chunk 0 (split by k-tile half so the first
        # 4 projection matmuls start after half the chunk-0 transfer)
        nc.sync.dma_start(out=wqkv_sb[:, :, 0:128], in_=wqkv_r[:, :, 0:128])
        nc.sync.dma_start(out=wqkv_sb[:, :, INC : INC + 128],
                          in_=wqkv_r[:, :, INC : INC + 128])
        nc.sync.dma_start(out=xt_sb[:, 0:4, 0:512], in_=xT_r[:, 0:4, 0:512])
        nc.sync.dma_start(out=xt_sb[:, 4:8, 0:512], in_=xT_r[:, 4:8, 0:512])
        nc.sync.dma_start(out=wqkv_sb[:, :, 2 * INC : 3 * INC],
                          in_=wqkv_r[:, :, 2 * INC : 3 * INC])
        nc.sync.dma_start(out=xt_sb[:, :, 512:1024], in_=xT_r[:, :, 512:1024])
        nc.sync.dma_start(out=xt_sb[:, :, 1024:1536], in_=xT_r[:, :, 1024:1536])
        nc.sync.dma_start(out=xt_sb[:, :, 1536:2048], in_=xT_r[:, :, 1536:2048])
        nc.sync.dma_start(out=wqkv_sb[:, :, 128:INC], in_=wqkv_r[:, :, 128:INC])
        nc.sync.dma_start(out=wqkv_sb[:, :, INC + 128 : 2 * INC],
                          in_=wqkv_r[:, :, INC + 128 : 2 * INC])
        wcqa_sb = consts.tile([128, 2 * DH], BF16, name="wcqa_sb")
        nc.sync.dma_start(out=wcqa_sb[0:DH, :], in_=wcq_d)
        nc.sync.dma_start(out=wcqa_sb[DH:128, :], in_=wca_d)

        # ACT table warmup: exp+tanh share one set; nothing else is used.
        warm_sb = consts.tile([128, 1], F32, name="warm_sb")
        nc.scalar.activation(out=warm_sb, in_=bias_sb, func=AF.Exp)
        nc.scalar.activation(out=warm_sb, in_=warm_sb, func=AF.Tanh)

        # ---------------- projection units (as filler micros) ----------------
        def qk_micros(wsb, p, c, is_q):
            """8 matmuls + evacuation, one closure per instruction group."""
            state = {}

            def mk_mm(k):
                def _mm():
                    if "ps" not in state:
                        state["ps"] = psum.tile(
                            [128, 512], F32, name="ps_qk", tag="spair", bufs=3
                        )
                    nc.tensor.matmul(
                        state["ps"],
                        lhsT=wsb[:, k, p * 128 : (p + 1) * 128],
                        rhs=xt_sb[:, k, c * 512 : (c + 1) * 512],
                        start=(k == 0),
                        stop=(k == KT - 1),
                    )
                return (260, _mm)

            def _evac_pair():
                dst = qTc[p][c] if is_q else kTc[p][c]
                nc.vector.tensor_copy(out=dst, in_=state["ps"])

            if not is_q:
                return [mk_mm(k) for k in range(KT)] + [(60, _evac_pair)]

            def _evac_qa():
                ps = state["ps"]
                nc.vector.tensor_copy(out=qac[2 * p][c][0:DH, :], in_=ps[0:DH, :])
                nc.vector.tensor_copy(out=qac[2 * p + 1][c][0:DH, :],
                                      in_=ps[DH:128, :])

            return [mk_mm(k) for k in range(KT)] + [(60, _evac_pair),
                                                    (60, _evac_qa)]

        def v_micros(it):
            state = {}

            def mk_mm(k):
                def _mm():
                    if "ps" not in state:
                        state["ps"] = psum.tile(
                            [128, 512], F32, name="ps_v", tag="spair", bufs=3
                        )
                    nc.tensor.matmul(
                        state["ps"][:, 0:INC],
                        lhsT=xt_sb[:, k, it * 128 : (it + 1) * 128],
                        rhs=wv_sb[:, k, :],
                        start=(k == 0),
                        stop=(k == KT - 1),
                    )
                return (150, _mm)

            def _evac0():
                for p2 in range(2):
                    nc.vector.tensor_copy(
                        out=varena[it][:, p2, 0, 0:DH],
                        in_=state["ps"][:, (2 * p2) * DH : (2 * p2 + 1) * DH],
                    )

            def _evac1():
                for p2 in range(2):
                    nc.vector.tensor_copy(
                        out=varena[it][:, p2, 1, DH:128],
                        in_=state["ps"][:, (2 * p2 + 1) * DH : (2 * p2 + 2) * DH],
                    )

            return [mk_mm(k) for k in range(KT)] + [(60, _evac0), (60, _evac1)]

        # ---------------- filler queue (deadline order) ----------------
        # Emission order IS dependency order for the tile framework, so a
        # consumer (S/PV/AoA) must be emitted after its producer unit. The
        # `ensure` calls force-drain fillers up to the labeled unit; the
        # budget metering only paces additional prefetch.
        fillers = deque()
        emitted_units = set()

        def push_unit(ms, label):
            ms = list(ms)
            for cost, fn in ms[:-1]:
                fillers.append((cost, fn, None))
            cost, fn = ms[-1]
            fillers.append((cost, fn, label))

        def pop_filler():
            cost, fn, label = fillers.popleft()
            fn()
            if label is not None:
                emitted_units.add(label)
            return cost

        def ensure(label):
            while fillers and label not in emitted_units:
                pop_filler()

        # prefix (emitted immediately): q(p0,c0), k(p0,c0), v tiles 0-5
        for cost, fn in qk_micros(wq_sb, 0, 0, True):
            fn()
        for cost, fn in qk_micros(wk_sb, 0, 0, False):
            fn()
        for it in range(4):
            for cost, fn in v_micros(it):
                fn()

        push_unit(v_micros(4), ("v", 4))
        push_unit(v_micros(5), ("v", 5))
        push_unit(qk_micros(wk_sb, 0, 1, False), ("k", 0, 1))
        push_unit(v_micros(6), ("v", 6))
        push_unit(v_micros(7), ("v", 7))
        push_unit(qk_micros(wk_sb, 0, 2, False), ("k", 0, 2))
        push_unit(v_micros(8), ("v", 8))
        push_unit(v_micros(9), ("v", 9))
        push_unit(qk_micros(wk_sb, 0, 3, False), ("k", 0, 3))
        push_unit(qk_micros(wq_sb, 0, 1, True), ("q", 0, 1))
        push_unit(v_micros(10), ("v", 10))
        push_unit(v_micros(11), ("v", 11))
        push_unit(v_micros(12), ("v", 12))
        push_unit(qk_micros(wq_sb, 0, 2, True), ("q", 0, 2))
        push_unit(v_micros(13), ("v", 13))
        push_unit(v_micros(14), ("v", 14))
        push_unit(v_micros(15), ("v", 15))
        push_unit(qk_micros(wq_sb, 0, 3, True), ("q", 0, 3))
        push_unit(qk_micros(wk_sb, 1, 0, False), ("k", 1, 0))
        push_unit(qk_micros(wq_sb, 1, 0, True), ("q", 1, 0))
        push_unit(qk_micros(wk_sb, 1, 1, False), ("k", 1, 1))
        push_unit(qk_micros(wk_sb, 1, 2, False), ("k", 1, 2))
        push_unit(qk_micros(wk_sb, 1, 3, False), ("k", 1, 3))
        push_unit(qk_micros(wq_sb, 1, 1, True), ("q", 1, 1))
        push_unit(qk_micros(wq_sb, 1, 2, True), ("q", 1, 2))
        push_unit(qk_micros(wq_sb, 1, 3, True), ("q", 1, 3))

        # ---------------- per-chunk boundary work ----------------
        def mk_norm(p, c, pv_state, emit_pv):
            """Carry items riding chunk n+1: finish PV, evacuate ao, and
            normalize using the free L rows (pvA row 64 / pvB row 63)."""
            items = []

            def _pvlast():
                ensure(("v", JT - 1))
                emit_pv(JT - 2)
                emit_pv(JT - 1)
            items.append(_pvlast)

            lst = {}

            def mk_evac(hh):
                def _evac():
                    # ao + L row to SBUF at base partition 0 (DVE copies may
                    # shift partitions); frees the pv bank for the next chunk
                    ao = aop.tile([DH, 512], BF16, name=f"ao{hh}",
                                  tag=f"ao{hh}")
                    lst[f"ao{hh}"] = ao
                    l0 = nrm.tile([1, 512], F32, name=f"l0_{hh}",
                                  tag=f"l0{hh}")
                    lst[f"l0{hh}"] = l0
                    if hh == 0:
                        nc.vector.tensor_copy(out=ao,
                                              in_=pv_state["pvA"][0:DH, :])
                        nc.vector.tensor_copy(
                            out=l0, in_=pv_state["pvA"][DH : DH + 1, :])
                    else:
                        nc.vector.tensor_copy(out=ao,
                                              in_=pv_state["pvB"][DH:128, :])
                        nc.vector.tensor_copy(
                            out=l0, in_=pv_state["pvB"][32:33, :])
                return _evac
            items.append(mk_evac(0))
            items.append(mk_evac(1))

            def mk_lrec(hh):
                def _lrec():
                    rl = nrm.tile([1, 512], F32, name="rl", tag=f"rl{hh}")
                    nc.vector.reciprocal_approx_fast(out=rl,
                                                     in_=lst[f"l0{hh}"])
                    rlb = nrm.tile([DH, 512], F32, name="rlb", tag=f"rlb{hh}")
                    lst[f"rlb{hh}"] = rlb
                    nc.gpsimd.partition_broadcast(rlb, rl)
                return _lrec
            items.append(mk_lrec(0))
            items.append(mk_lrec(1))

            def mk_mul(hh):
                def _mul():
                    # normalized ao -> rows 64:128 of qac (DVE)
                    nc.vector.tensor_mul(
                        out=qac[2 * p + hh][c][DH:128, :],
                        in0=lst[f"ao{hh}"],
                        in1=lst[f"rlb{hh}"],
                    )
                return _mul
            items.append(mk_mul(0))
            items.append(mk_mul(1))
            return items

        def mk_finals_items(p, c):
            """Carry items riding chunk n+2: AoA + tanh gate + output."""
            items = []

            def mk_finals(hh):
                h = 2 * p + hh
                cs = slice(c * 512, (c + 1) * 512)
                fst = {}

                def _aoa():
                    fst["ig"] = psum.tile([128, 512], F32, name="igh",
                                          tag="spair", bufs=3)
                    nc.tensor.matmul(fst["ig"], lhsT=wcqa_sb, rhs=qac[h][c],
                                     start=True, stop=True)

                def _tanh():
                    t = fin.tile([DH, 512], BF16, name="tgate", tag="t")
                    fst["t"] = t
                    nc.scalar.activation(
                        out=t, in_=fst["ig"][DH:128, :], func=AF.Tanh,
                        scale=0.5, bias=bias_sb[DH : 2 * DH, :],
                    )

                def _stt1():
                    # P = (0.5*I_raw + 0.5*b_out) * t   (weights/bias pre-halved)
                    pt = fin.tile([DH, 512], BF16, name="pt", tag="pt")
                    fst["pt"] = pt
                    nc.vector.scalar_tensor_tensor(
                        out=pt, in0=fst["ig"][0:DH, :],
                        scalar=bias_sb[0:DH, :], in1=fst["t"],
                        op0=ALU.add, op1=ALU.mult,
                    )

                def _stt2():
                    # out = (0.5*I_raw + 0.5*b_out) + P  = (I+b)*sigmoid(G+bg)
                    ot = fin.tile([DH, 512], BF16, name="ot", tag="ot")
                    nc.vector.scalar_tensor_tensor(
                        out=ot, in0=fst["ig"][0:DH, :],
                        scalar=bias_sb[0:DH, :], in1=fst["pt"],
                        op0=ALU.add, op1=ALU.add,
                    )
                    nc.sync.dma_start(out=outT_d[h * DH : (h + 1) * DH, cs],
                                      in_=ot)

                return [_aoa, _tanh, _stt1, _stt2]

            items.extend(mk_finals(0))
            items.extend(mk_finals(1))
            return items

        # ---------------- the stream ----------------
        carry = []
        pending_finals = []
        chunk_idx = 0
        for p in range(2):
            for c in range(NCH):
                pv_state = {}
                es_tiles = [None] * JT

                def emit_pv(jt, p=p, pv_state=pv_state, es_tiles=es_tiles):
                    # Full-array 128-weight-col matmuls (FWL, ~2 cols/cycle).
                    # Bank A rows 0:64 = ao_A, row 64 = L_A (ones col);
                    # bank B row 63 = L_B, rows 64:128 = ao_B. The zero-
                    # padded weight cols make the other rows harmless.
                    if "pvA" not in pv_state:
                        pv_state["pvA"] = psum.tile([128, 512], F32, name="pvA",
                                                    tag="pv", bufs=2)
                        pv_state["pvB"] = psum.tile([128, 512], F32, name="pvB",
                                                    tag="pv", bufs=2)
                    nc.tensor.matmul(
                        pv_state["pvA"], lhsT=varena[jt][:, p, 0, :],
                        rhs=es_tiles[jt][:, 0:512],
                        start=(jt == 0), stop=(jt == JT - 1),
                    )
                    nc.tensor.matmul(
                        pv_state["pvB"], lhsT=varena[jt][:, p, 1, :],
                        rhs=es_tiles[jt][:, 512:1024],
                        start=(jt == 0), stop=(jt == JT - 1),
                    )

                budget = 0.0
                if (p, c) != (0, 0):
                    ensure(("q", p, c))
                for jt in range(JT):
                    if p > 0 or jt // 4 > 0:
                        ensure(("k", p, jt // 4))
                    jts = slice((jt % 4) * 128, (jt % 4) * 128 + 128)
                    kt_t = kTc[p][jt // 4]
                    s = psum.tile([128, 1024], F32, name="s", tag="spair",
                                  bufs=3)
                    nc.tensor.matmul(
                        s[:, 0:512], lhsT=kt_t[0:DH, jts], rhs=qTc[p][c][0:DH, :],
                        start=True, stop=True, tile_position=(0, 0),
                    )
                    nc.tensor.matmul(
                        s[:, 512:1024], lhsT=kt_t[DH:128, jts],
                        rhs=qTc[p][c][DH:128, :],
                        start=True, stop=True, tile_position=(64, 0),
                    )
                    es = esp.tile([128, 1024], BF16, name="es", tag="es")
                    nc.scalar.activation(out=es, in_=s, func=AF.Exp, scale=SCALE)
                    es_tiles[jt] = es
                    # earlier chunks' boundary work, one item per step:
                    # norm of chunk n-1 (even steps) / finals of chunk n-2
                    # (odd steps) -- interleaved so every chain hop gets ~2
                    # steps of slack and never blocks the in-order PE queue
                    if carry:
                        carry.pop(0)()
                    # PV lags 2 steps so the pv-bank evac of the previous
                    # chunk (carry item at step 2) clears before PV(0) needs
                    # the bank
                    if jt > 1:
                        if jt - 2 > 3:
                            ensure(("v", jt - 2))
                        emit_pv(jt - 2)
                    # projection fillers, cost-metered
                    budget += FILL_EARLY if chunk_idx == 0 else FILL
                    while fillers and fillers[0][0] <= budget:
                        budget -= pop_filler()

                while carry:
                    carry.pop(0)()
                # two-deep carry: norm (pv finish/evac/1/L/qac write) rides
                # the NEXT chunk on even steps; finals (AoA/tanh/output) of
                # the chunk before ride on odd steps. Every item's inputs
                # are >=2 steps old when it enters its engine queue.
                norm_items = mk_norm(p, c, pv_state, emit_pv)
                carry = []
                for i in range(max(len(norm_items), len(pending_finals))):
                    if i < len(norm_items):
                        carry.append(norm_items[i])
                    if i < len(pending_finals):
                        carry.append(pending_finals[i])
                pending_finals = mk_finals_items(p, c)
                chunk_idx += 1

        # ---------------- tail: flush the last chunks ----------------
        while fillers:
            pop_filler()
        while carry:
            carry.pop(0)()
        for it in pending_finals:
            it()
    nc.compile()
    return nc


_NC_CACHE = None


def _get_nc():
    global _NC_CACHE
    if _NC_CACHE is None:
        _NC_CACHE = build_nc()
    return _NC_CACHE


def make_in_maps(x, Wq, Wkv, Wq_out, Wattn_out, out_bias, Wq_gate, Wattn_gate,
                 gate_bias):
    import ml_dtypes

    bf16 = ml_dtypes.bfloat16
    # I-columns (first DH) pre-halved for the tanh-form gate; biases halved.
    wcq = np.concatenate([0.5 * Wq_out.T, Wq_gate.T], axis=1)
    wcq = np.ascontiguousarray(wcq, dtype=bf16)
    wca = np.concatenate([0.5 * Wattn_out.T, Wattn_gate.T], axis=1)
    wca = np.ascontiguousarray(wca, dtype=bf16)
    biases = 0.5 * np.concatenate(
        [out_bias.reshape(-1), gate_bias.reshape(-1)]
    ).astype(np.float32).reshape(2 * DH, 1)
    biases = np.ascontiguousarray(biases)
    Wk = Wkv[:, : H * DH]
    Wv = Wkv[:, H * DH :]
    xT = [
        np.ascontiguousarray(x[b].T.reshape(KT, 128, N)).astype(bf16)
        for b in range(B)
    ]
    in_maps = []
    for c in range(8):
        b, hg = c // 4, c % 4
        cols = slice(hg * INC, (hg + 1) * INC)
        in_maps.append(
            {
                "xT": xT[b],
                "wqkv": np.ascontiguousarray(
                    np.concatenate(
                        [Wq[:, cols], Wk[:, cols], Wv[:, cols]], axis=1
                    )
                ).astype(bf16),
                "wcq": wcq,
                "wca": wca,
                "biases": biases,
            }
        )
    return in_maps


def assemble_output(results):
    out = np.empty((B, N, H * DH), dtype=np.float32)
    for c in range(8):
        b, hg = c // 4, c % 4
        out[b, :, hg * INC : (hg + 1) * INC] = results[c]["outT"].T.astype(np.float32)
    return out


def kernel(**inputs):
    from concourse.bass_utils import run_bass_kernel_spmd

    inputs = {k: np.asarray(v, dtype=np.float32) for k, v in inputs.items()}
    nc = _get_nc()
    in_maps = make_in_maps(**inputs)
    res = run_bass_kernel_spmd(nc, in_maps, core_ids=list(range(8)))
    return assemble_output(res.results)
